# revision 44
# baseline (speedup 1.0000x reference)
"""Trainium2 Bass kernel for the additive-attention layer.

Math (per batch b):
    pre[s, h]   = enc[b] @ W2 + hidden[b] @ W1 + b_attn      (W1=W_attn[:H], W2=W_attn[H:])
    energy      = tanh(pre)
    scores[s]   = energy @ w_v (+ b_v, irrelevant: softmax is shift-invariant)
    attn        = softmax(scores)
    context     = attn @ enc[b]

Distribution: data-parallel over batch, 4 batches per core, no collectives.

Device dataflow per (batch, s-chunk of 512):
  - encT tiles (d on partitions, s free) arrive via DMA; the host pre-tiles
    to (b, c, p, k, s) so each chunk DMA is 16KB-contiguous per partition.
  - TensorE: psum[h128, s512] += W2[d128, h128].T @ encT[d128, s512]  (16 d-tiles)
  - ScalarE: energy = tanh(psum + hproj[b]) -> SBUF bf16   (hproj = W1.T@hidden + b_attn)
  - TensorE: scores_psum[1, s512] += w_v[h128, 1].T @ energy  (8 h-tiles)
  - ScalarE: p = exp(scores) -> attn row, accum_out = chunk denominator
  - GpSimd: broadcast p across 128 partitions
  - ctx partials: prod = encT * p on DVE; free-dim reduce split between DVE
    tensor_reduce (k < DVE_SPLIT) and ACT Identity accum (rest) so neither
    engine's FIFO backs up and gates PSUM-bank release for the next chunk.
Finalize is per-batch, emitted right after that batch's last chunk so it
overlaps the next batch's main loop; only the last batch's ~3us trails.
"""

import numpy as np
import ml_dtypes
from contextlib import ExitStack

import concourse.bacc as bacc
import concourse.bass as bass
import concourse.tile as tile
import concourse.mybir as mybir
from concourse.bass_utils import run_bass_kernel_spmd

B, S, H = 32, 2048, 1024
D = 2 * H                     # encoder feature dim
NCORES = 8
BPC = B // NCORES             # batches per core
SCH = 512                     # s-chunk (one PSUM bank of fp32)
NCH = S // SCH
NDT = D // 128                # d-tiles (contraction tiles for main matmul)
NHT = H // 128                # h-tiles
NKT = H // 128                # d-tiles for the W1 projection
KG = 4                        # k-tiles per startup sub-DMA group

BF16 = mybir.dt.bfloat16
F32 = mybir.dt.float32

_CACHE = {}


def _build(encp_bufs=4, enp_bufs=12, ppre_bufs=6, psc_bufs=1, scr_bufs=6,
           dve_split=6):
    nc = bacc.Bacc("TRN2", target_bir_lowering=False, debug=False)

    # Host pre-tiles everything into partition-major layouts so every
    # device DMA is one contiguous run per partition (big descriptors).
    encT = nc.dram_tensor("enct", (BPC, NCH, 128, NDT, SCH), BF16,
                          kind="ExternalInput").ap()
    w2 = nc.dram_tensor("w2", (128, NDT, H), BF16, kind="ExternalInput").ap()
    # w1 j-major so hproj j-groups can start after 1MB of DMA
    w1 = nc.dram_tensor("w1", (128, NHT, NKT, 128), BF16, kind="ExternalInput").ap()
    hidT = nc.dram_tensor("hidt", (128, NKT, BPC), BF16, kind="ExternalInput").ap()
    wv = nc.dram_tensor("wv", (128, NHT), BF16, kind="ExternalInput").ap()
    ba = nc.dram_tensor("ba", (128, NHT), F32, kind="ExternalInput").ap()
    # ctx in partition-major (b, p, k); host untransposes
    ctx_out = nc.dram_tensor("ctx", (BPC, 128, NDT), F32, kind="ExternalOutput").ap()
    attn_out = nc.dram_tensor("attn", (BPC, S), F32, kind="ExternalOutput").ap()

    with tile.TileContext(nc) as tc, ExitStack() as ctx:
        weights = ctx.enter_context(tc.tile_pool(name="weights", bufs=1))
        encp = ctx.enter_context(tc.tile_pool(name="encp", bufs=encp_bufs))
        enp = ctx.enter_context(tc.tile_pool(name="enp", bufs=enp_bufs))
        small = ctx.enter_context(tc.tile_pool(name="small", bufs=1))
        bcp = ctx.enter_context(tc.tile_pool(name="bcp", bufs=4))
        scr = ctx.enter_context(tc.tile_pool(name="scr", bufs=scr_bufs))
        outp = ctx.enter_context(tc.tile_pool(name="outp", bufs=2))
        ppre = ctx.enter_context(tc.tile_pool(name="ppre", bufs=ppre_bufs, space="PSUM"))
        psc = ctx.enter_context(tc.tile_pool(name="psc", bufs=psc_bufs, space="PSUM"))
        pmisc = ctx.enter_context(tc.tile_pool(name="pmisc", bufs=1, space="PSUM"))

        # --- startup DMAs ---
        # Each dma_start occupies its HWDGE issue queue ~660ns, so the
        # startup is ISSUE-paced: w2 pieces go on the Sync ring and enc
        # pieces on the Scalar ring (both HWDGE) to issue in parallel,
        # and the small aux loads (hidT/wv/ba, needed ~24us in) plus w1
        # ride behind the critical k-stream.
        w1_sb = weights.tile([128, NHT, NKT, 128], BF16)
        w2_sb = weights.tile([128, NDT, H], BF16)
        et0 = encp.tile([128, NDT, SCH], BF16, name="et0", tag="et")
        hidT_sb = small.tile([128, NKT, BPC], BF16)
        wv_sb = small.tile([128, NHT], BF16)
        ba_sb = small.tile([128, NHT], F32)
        # single-k first pieces so chunk 0's first matmul starts ASAP
        for k in range(2):
            nc.sync.dma_start(out=w2_sb[:, k:k + 1, :], in_=w2[:, k:k + 1, :])
            nc.scalar.dma_start(out=et0[:, k:k + 1, :],
                                in_=encT[0, 0, :, k:k + 1, :])
        for kk in range(1, NDT // 2):
            nc.sync.dma_start(out=w2_sb[:, kk * 2:kk * 2 + 2, :],
                              in_=w2[:, kk * 2:kk * 2 + 2, :])
            nc.scalar.dma_start(out=et0[:, kk * 2:kk * 2 + 2, :],
                                in_=encT[0, 0, :, kk * 2:kk * 2 + 2, :])
            if kk == 3:
                nc.sync.dma_start(out=hidT_sb, in_=hidT)
                nc.scalar.dma_start(out=ba_sb, in_=ba)
            if kk == 5:
                nc.scalar.dma_start(out=w1_sb[:, 0:4], in_=w1[:, 0:4])
                nc.sync.dma_start(out=wv_sb, in_=wv)
            if kk == 7:
                nc.scalar.dma_start(out=w1_sb[:, 4:8], in_=w1[:, 4:8])

        # --- persistent accumulators ---
        hproj = small.tile([128, NHT, BPC], F32, name="hproj", tag="hproj")
        attn_rows = [
            small.tile([1, S], F32, name=f"attnrow{b}", tag=f"attnrow{b}")
            for b in range(BPC)
        ]
        # +1 col: the last chunk is split into two s-halves (cols 15, 16)
        denp = small.tile([1, BPC * NCH + 1], F32, name="denp", tag="denp")
        # context partials: column layout (b, k, c); the split last chunk
        # gets its own (k, h) block
        ctxp = small.tile([128, BPC * NDT * NCH], F32, name="ctxp", tag="ctxp")
        ctxp3 = small.tile([128, NDT * 2], F32, name="ctxp3", tag="ctxp3")

        # scores accumulate col-tiled into one persistent PSUM bank
        # (partitions 0/32/64/96); rest of the bank stays memset-zero.
        # One bank is enough: consecutive chunks' scores are ~30us apart.
        pscb0 = psc.tile([128, SCH], F32, name="pscb0", tag="pscb0")
        nc.vector.memset(pscb0, 0.0)
        ones4 = small.tile([128, 1], BF16, name="ones4", tag="ones4")
        nc.vector.memset(ones4, 0.0)
        for q in range(4):
            nc.vector.memset(ones4[32 * q:32 * q + 1], 1.0)
        # row-of-ones for PE partition-broadcast in the kernel tail
        ones_r = small.tile([1, 128], BF16, name="ones_r", tag="ones_r")
        nc.vector.memset(ones_r, 1.0)
        ones_rf = small.tile([1, 128], F32, name="ones_rf", tag="ones_rf")
        nc.vector.memset(ones_rf, 1.0)

        # --- hproj[h, b] = W1.T @ hidden.T + b_attn ---
        # j 0..3 before the main loop; j 4..7 between chunk 0's halves so
        # chunk 0's first matmuls aren't queued behind the w1 jg1 DMA
        def emit_hproj(j0, j1):
            for j in range(j0, j1):
                ph = pmisc.tile([128, BPC], F32, name=f"ph{j}", tag="pm")
                for k in range(NKT):
                    nc.tensor.matmul(
                        ph,
                        w1_sb[:, j, k, :],
                        hidT_sb[:, k, :],
                        start=(k == 0),
                        stop=(k == NKT - 1),
                    )
                nc.scalar.activation(
                    out=hproj[:, j, :],
                    in_=ph,
                    func=mybir.ActivationFunctionType.Identity,
                    bias=ba_sb[:, j:j + 1],
                    scale=1.0,
                )

        # (hproj emission is deferred into chunk 0's halves below so the PE
        # FIFO reaches chunk 0's matmuls as soon as the first k-pair lands)

        def emit_scores_ctx(b, c, et, soff, sl, dcol, colf, energies, last):
            """scores + exp + ctx partials for s-slice [soff,soff+sl) of
            chunk (b,c). colf(k) -> ctx partial column AP."""
            # scores: 8 col-tiled rank-1 matmuls (4 column groups run
            # concurrently), partial rows at partitions 0/32/64/96,
            # then one ones-matmul sums the rows
            pscb = pscb0
            for j in range(NHT):
                q = j % 4
                # has_written clear is per-element: each column group's
                # row needs its own start (j<4) / stop (j>=4)
                nc.tensor.matmul(
                    pscb[32 * q:32 * q + 1, soff:soff + sl],
                    wv_sb[:, j:j + 1],
                    energies[j],
                    start=(j < 4),
                    stop=(j >= 4),
                    tile_position=(0, 32 * q),
                    skip_group_check=True,
                )
            scp = bcp.tile([128, sl], BF16, name=f"scp{b}_{c}_{soff}",
                           tag="scp")
            nc.vector.tensor_copy(scp, pscb[:, soff:soff + sl])
            ps = pmisc.tile([1, sl], F32, name=f"ps_{b}_{c}_{soff}", tag="pm")
            nc.tensor.matmul(ps, ones4, scp, start=True, stop=True)

            prow = attn_rows[b][0:1, c * SCH + soff:c * SCH + soff + sl]
            nc.scalar.activation(
                out=prow,
                in_=ps,
                func=mybir.ActivationFunctionType.Exp,
                accum_out=denp[0:1, dcol:dcol + 1],
            )

            # broadcast p across partitions, then per-d-tile mul+reduce
            prow_bf = bcp.tile([1, sl], BF16, name=f"pbf{b}_{c}_{soff}",
                               tag="prow_bf")
            nc.scalar.activation(
                out=prow_bf,
                in_=prow,
                func=mybir.ActivationFunctionType.Identity,
            )
            pbc = bcp.tile([128, sl], BF16, name=f"pbc{b}_{c}_{soff}",
                           tag="pbc")
            if last:
                # tail: PE is idle and GpSimd pays a drain between
                # broadcasts — broadcast via ones-matmul instead
                psbc = pmisc.tile([128, sl], F32, name=f"psb{b}_{c}_{soff}",
                                  tag="pm")
                nc.tensor.matmul(psbc, ones_r, prow_bf, start=True, stop=True)
                nc.vector.tensor_copy(pbc, psbc)
            else:
                nc.gpsimd.partition_broadcast(pbc, prow_bf)

            for k in range(NDT):
                prod = scr.tile([128, sl], BF16, name=f"pr{b}_{c}_{soff}_{k}",
                                tag="prod")
                nc.vector.tensor_mul(prod, et[:, k, soff:soff + sl], pbc)
                # last chunk trails the kernel: alternate engines so
                # DVE and ACT drain its reduces concurrently
                if (k % 2 == 0) if last else (k < dve_split):
                    nc.vector.tensor_reduce(
                        colf(k),
                        prod,
                        axis=mybir.AxisListType.X,
                        op=mybir.AluOpType.add,
                    )
                else:
                    prod2 = scr.tile([128, sl], BF16,
                                     name=f"p2{b}_{c}_{soff}_{k}", tag="prod2")
                    nc.scalar.activation(
                        out=prod2,
                        in_=prod,
                        func=mybir.ActivationFunctionType.Identity,
                        accum_out=colf(k),
                    )

        def emit_body(b, c, et, soff, sl, dcol, colf, last):
            energies = []
            for j in range(NHT):
                pp = ppre.tile([128, sl], F32, name=f"pp{b}_{c}_{soff}_{j}",
                               tag="pp")
                for k in range(NDT):
                    nc.tensor.matmul(
                        pp,
                        w2_sb[:, k, j * 128:(j + 1) * 128],
                        et[:, k, soff:soff + sl],
                        start=(k == 0),
                        stop=(k == NDT - 1),
                    )
                en = enp.tile([128, sl], BF16, name=f"en{b}_{c}_{soff}_{j}",
                              tag="en")
                nc.scalar.activation(
                    out=en,
                    in_=pp,
                    func=mybir.ActivationFunctionType.Tanh,
                    bias=hproj[:, j, b:b + 1],
                    scale=1.0,
                )
                energies.append(en)
            emit_scores_ctx(b, c, et, soff, sl, dcol, colf, energies, last)

        # --- main loop ---
        for b in range(BPC):
            for c in range(NCH):
                if b == 0 and c == 0:
                    et = et0
                else:
                    et = encp.tile([128, NDT, SCH], BF16, tag="et")
                    nc.sync.dma_start(out=et, in_=encT[b, c])

                def colf_std(k, b=b, c=c):
                    col = (b * NDT + k) * NCH + c
                    return ctxp[:, col:col + 1]

                if b == 0 and c == 0:
                    # k-outer passes sized so PE consumption (~1.3us/k for
                    # 6 j's) matches the startup DMA arrival rate (~1.4us/k)
                    energies = [None] * NHT
                    for j0, j1 in ((0, 6), (6, 8)):
                        js = range(j0, j1)
                        pps = {j: ppre.tile([128, SCH], F32, name=f"pp0_{j}",
                                            tag="pp") for j in js}
                        for k in range(NDT):
                            for j in js:
                                nc.tensor.matmul(
                                    pps[j],
                                    w2_sb[:, k, j * 128:(j + 1) * 128],
                                    et[:, k, :],
                                    start=(k == 0),
                                    stop=(k == NDT - 1),
                                )
                        if j0 == 0:
                            emit_hproj(0, 8)
                        for j in js:
                            en = enp.tile([128, SCH], BF16, name=f"en00_{j}",
                                          tag="en")
                            nc.scalar.activation(
                                out=en,
                                in_=pps[j],
                                func=mybir.ActivationFunctionType.Tanh,
                                bias=hproj[:, j, b:b + 1],
                                scale=1.0,
                            )
                            energies[j] = en
                    emit_scores_ctx(b, c, et, 0, SCH, b * NCH + c,
                                    colf_std, energies, False)
                elif b == BPC - 1 and c == NCH - 1:
                    # final chunk as two s-halves: the first half's softmax/
                    # ctx pipeline drains under the second half's matmuls
                    for h in range(2):
                        def colf3(k, h=h):
                            return ctxp3[:, k * 2 + h:k * 2 + h + 1]
                        emit_body(b, c, et, h * (SCH // 2), SCH // 2,
                                  BPC * NCH - 1 + h, colf3, True)
                else:
                    emit_body(b, c, et, 0, SCH, b * NCH + c, colf_std, False)

            # --- per-batch finalize, overlaps next batch's chunks ---
            lastb = b == BPC - 1
            dent = outp.tile([1, 1], F32, tag="dent")
            nc.vector.tensor_reduce(
                dent,
                denp[0:1, b * NCH:(b + 1) * NCH + (1 if lastb else 0)],
                axis=mybir.AxisListType.X,
                op=mybir.AluOpType.add,
            )
            rv = outp.tile([1, 1], F32, tag="rv")
            nc.vector.reciprocal(rv, dent)

            attn_f = outp.tile([1, S], F32, tag="attnf")
            nc.vector.tensor_scalar_mul(attn_f, attn_rows[b], rv)
            nc.sync.dma_start(out=attn_out[b:b + 1, :], in_=attn_f)

            ctxr = outp.tile([128, NDT], F32, tag="ctxr")
            if lastb:
                # last batch: c 0..2 from ctxp, split chunk from ctxp3
                r1 = outp.tile([128, NDT], F32, tag="ctxr1")
                nc.vector.tensor_reduce(
                    r1,
                    ctxp.rearrange("p (x c) -> p x c", c=NCH)[
                        :, b * NDT:(b + 1) * NDT, 0:NCH - 1],
                    axis=mybir.AxisListType.X,
                    op=mybir.AluOpType.add,
                )
                r2 = outp.tile([128, NDT], F32, tag="ctxr2")
                nc.vector.tensor_reduce(
                    r2,
                    ctxp3.rearrange("p (x h) -> p x h", h=2),
                    axis=mybir.AxisListType.X,
                    op=mybir.AluOpType.add,
                )
                nc.vector.tensor_add(ctxr, r1, r2)
            else:
                nc.vector.tensor_reduce(
                    ctxr,
                    ctxp.rearrange("p (x c) -> p x c", c=NCH)[
                        :, b * NDT:(b + 1) * NDT, :],
                    axis=mybir.AxisListType.X,
                    op=mybir.AluOpType.add,
                )
            rvb = outp.tile([128, 1], F32, tag="rvb")
            if lastb:
                rvp = pmisc.tile([128, 1], F32, name="rvp", tag="pm")
                nc.tensor.matmul(rvp, ones_rf, rv, start=True, stop=True)
                nc.vector.tensor_copy(rvb, rvp)
            else:
                nc.gpsimd.partition_broadcast(rvb, rv)
            ctxf = outp.tile([128, NDT], F32, tag="ctxf")
            nc.vector.tensor_scalar_mul(ctxf, ctxr, rvb)
            nc.sync.dma_start(out=ctx_out[b], in_=ctxf)

    nc.compile()
    return nc


def _get_nc():
    if "nc" not in _CACHE:
        _CACHE["nc"] = _build()
    return _CACHE["nc"]


def _prep_inputs(hidden, encoder_outputs, W_attn, b_attn, w_v, b_v):
    bf16 = ml_dtypes.bfloat16
    # W1 (H,H) -> (128, NKT, H): partition p holds rows p, p+128, ...
    # W1 (H,H) -> (128, NHT, NKT, 128): [k*128+p, j*128+col] -> [p, j, k, col]
    w1 = np.ascontiguousarray(
        W_attn[:H].reshape(NKT, 128, NHT, 128).transpose(1, 2, 0, 3)).astype(bf16)
    w2 = np.ascontiguousarray(
        W_attn[H:].reshape(NDT, 128, H).transpose(1, 0, 2)).astype(bf16)
    wv_ = np.ascontiguousarray(w_v.reshape(NHT, 128).T).astype(bf16)
    ba_ = np.ascontiguousarray(
        np.asarray(b_attn, dtype=np.float32).reshape(NHT, 128).T)
    enc_bf = encoder_outputs.astype(bf16)  # cast first (fast)
    in_maps = []
    for core in range(NCORES):
        sl = slice(core * BPC, (core + 1) * BPC)
        # (b, s, d) -> (b, c, p, k, s): s = c*SCH + s', d = k*128 + p
        encT = np.ascontiguousarray(
            enc_bf[sl].reshape(BPC, NCH, SCH, NDT, 128).transpose(0, 1, 4, 3, 2))
        hidT = np.ascontiguousarray(
            hidden[sl].T.reshape(NKT, 128, BPC).transpose(1, 0, 2)).astype(bf16)
        in_maps.append(
            {
                "enct": encT,
                "w2": w2,
                "w1": w1,
                "hidt": hidT,
                "wv": wv_,
                "ba": ba_,
            }
        )
    return in_maps


def kernel(hidden, encoder_outputs, W_attn, b_attn, w_v, b_v, _trace=False):
    nc = _get_nc()
    in_maps = _prep_inputs(hidden, encoder_outputs, W_attn, b_attn, w_v, b_v)
    res = run_bass_kernel_spmd(
        nc, in_maps, core_ids=list(range(NCORES)), trace=_trace
    )
    # ctx arrives as (BPC, 128, NDT); d = k*128 + p
    context = np.concatenate(
        [r["ctx"].transpose(0, 2, 1).reshape(BPC, D) for r in res.results],
        axis=0)
    attn = np.concatenate([r["attn"] for r in res.results], axis=0)
    if _trace:
        _CACHE["last_results"] = res
    return context, attn


# revision 46
# speedup vs baseline: 1.0020x; 1.0020x over previous
"""Trainium2 Bass kernel for the additive-attention layer.

Math (per batch b):
    pre[s, h]   = enc[b] @ W2 + hidden[b] @ W1 + b_attn      (W1=W_attn[:H], W2=W_attn[H:])
    energy      = tanh(pre)
    scores[s]   = energy @ w_v (+ b_v, irrelevant: softmax is shift-invariant)
    attn        = softmax(scores)
    context     = attn @ enc[b]

Distribution: data-parallel over batch, 4 batches per core, no collectives.

Device dataflow per (batch, s-chunk of 512):
  - encT tiles (d on partitions, s free) arrive via DMA; the host pre-tiles
    to (b, c, p, k, s) so each chunk DMA is 16KB-contiguous per partition.
  - TensorE: psum[h128, s512] += W2[d128, h128].T @ encT[d128, s512]  (16 d-tiles)
  - ScalarE: energy = tanh(psum + hproj[b]) -> SBUF bf16   (hproj = W1.T@hidden + b_attn)
  - TensorE: scores_psum[1, s512] += w_v[h128, 1].T @ energy  (8 h-tiles)
  - ScalarE: p = exp(scores) -> attn row, accum_out = chunk denominator
  - GpSimd: broadcast p across 128 partitions
  - ctx partials: prod = encT * p on DVE; free-dim reduce split between DVE
    tensor_reduce (k < DVE_SPLIT) and ACT Identity accum (rest) so neither
    engine's FIFO backs up and gates PSUM-bank release for the next chunk.
Finalize is per-batch, emitted right after that batch's last chunk so it
overlaps the next batch's main loop; only the last batch's ~3us trails.
"""

import numpy as np
import ml_dtypes
from contextlib import ExitStack

import concourse.bacc as bacc
import concourse.bass as bass
import concourse.tile as tile
import concourse.mybir as mybir
from concourse.bass_utils import run_bass_kernel_spmd

B, S, H = 32, 2048, 1024
D = 2 * H                     # encoder feature dim
NCORES = 8
BPC = B // NCORES             # batches per core
SCH = 512                     # s-chunk (one PSUM bank of fp32)
NCH = S // SCH
NDT = D // 128                # d-tiles (contraction tiles for main matmul)
NHT = H // 128                # h-tiles
NKT = H // 128                # d-tiles for the W1 projection
KG = 4                        # k-tiles per startup sub-DMA group

BF16 = mybir.dt.bfloat16
F32 = mybir.dt.float32

_CACHE = {}


def _build(encp_bufs=4, enp_bufs=12, ppre_bufs=6, psc_bufs=1, scr_bufs=6,
           dve_split=6):
    nc = bacc.Bacc("TRN2", target_bir_lowering=False, debug=False)

    # Host pre-tiles everything into partition-major layouts so every
    # device DMA is one contiguous run per partition (big descriptors).
    encT = nc.dram_tensor("enct", (BPC, NCH, 128, NDT, SCH), BF16,
                          kind="ExternalInput").ap()
    w2 = nc.dram_tensor("w2", (128, NDT, H), BF16, kind="ExternalInput").ap()
    # w1 j-major so hproj j-groups can start after 1MB of DMA
    w1 = nc.dram_tensor("w1", (128, NHT, NKT, 128), BF16, kind="ExternalInput").ap()
    hidT = nc.dram_tensor("hidt", (128, NKT, BPC), BF16, kind="ExternalInput").ap()
    wv = nc.dram_tensor("wv", (128, NHT), BF16, kind="ExternalInput").ap()
    ba = nc.dram_tensor("ba", (128, NHT), F32, kind="ExternalInput").ap()
    # ctx in partition-major (b, p, k); host untransposes
    ctx_out = nc.dram_tensor("ctx", (BPC, 128, NDT), F32, kind="ExternalOutput").ap()
    attn_out = nc.dram_tensor("attn", (BPC, S), F32, kind="ExternalOutput").ap()

    with tile.TileContext(nc) as tc, ExitStack() as ctx:
        weights = ctx.enter_context(tc.tile_pool(name="weights", bufs=1))
        encp = ctx.enter_context(tc.tile_pool(name="encp", bufs=encp_bufs))
        enp = ctx.enter_context(tc.tile_pool(name="enp", bufs=enp_bufs))
        small = ctx.enter_context(tc.tile_pool(name="small", bufs=1))
        bcp = ctx.enter_context(tc.tile_pool(name="bcp", bufs=4))
        scr = ctx.enter_context(tc.tile_pool(name="scr", bufs=scr_bufs))
        outp = ctx.enter_context(tc.tile_pool(name="outp", bufs=2))
        ppre = ctx.enter_context(tc.tile_pool(name="ppre", bufs=ppre_bufs, space="PSUM"))
        psc = ctx.enter_context(tc.tile_pool(name="psc", bufs=psc_bufs, space="PSUM"))
        pmisc = ctx.enter_context(tc.tile_pool(name="pmisc", bufs=1, space="PSUM"))

        # --- startup DMAs, all on the Sync HWDGE ring (issue-paced at
        # ~660ns each): critical w2/enc k-pieces first; the aux loads
        # (hidT/wv/ba, consumers run ~24us in) and w1 ride behind.
        w1_sb = weights.tile([128, NHT, NKT, 128], BF16)
        w2_sb = weights.tile([128, NDT, H], BF16)
        et0 = encp.tile([128, NDT, SCH], BF16, name="et0", tag="et")
        hidT_sb = small.tile([128, NKT, BPC], BF16)
        wv_sb = small.tile([128, NHT], BF16)
        ba_sb = small.tile([128, NHT], F32)
        # single-k first pieces so chunk 0's first matmul starts ASAP
        for k in range(2):
            nc.sync.dma_start(out=w2_sb[:, k:k + 1, :], in_=w2[:, k:k + 1, :])
            nc.sync.dma_start(out=et0[:, k:k + 1, :],
                              in_=encT[0, 0, :, k:k + 1, :])
        for kk in range(1, NDT // 2):
            nc.sync.dma_start(out=w2_sb[:, kk * 2:kk * 2 + 2, :],
                              in_=w2[:, kk * 2:kk * 2 + 2, :])
            nc.sync.dma_start(out=et0[:, kk * 2:kk * 2 + 2, :],
                              in_=encT[0, 0, :, kk * 2:kk * 2 + 2, :])
            if kk == 3:
                nc.sync.dma_start(out=hidT_sb, in_=hidT)
                nc.sync.dma_start(out=ba_sb, in_=ba)
            # hproj runs after chunk 0's first pass (~24us in), so w1 rides
            # at the back of the critical k-stream
            if kk == 5:
                nc.sync.dma_start(out=w1_sb[:, 0:4], in_=w1[:, 0:4])
                nc.sync.dma_start(out=wv_sb, in_=wv)
            if kk == 7:
                nc.sync.dma_start(out=w1_sb[:, 4:8], in_=w1[:, 4:8])

        # --- persistent accumulators ---
        hproj = small.tile([128, NHT, BPC], F32, name="hproj", tag="hproj")
        attn_rows = [
            small.tile([1, S], F32, name=f"attnrow{b}", tag=f"attnrow{b}")
            for b in range(BPC)
        ]
        # +1 col: the last chunk is split into two s-halves (cols 15, 16)
        denp = small.tile([1, BPC * NCH + 1], F32, name="denp", tag="denp")
        # context partials: column layout (b, k, c); the split last chunk
        # gets its own (k, h) block
        ctxp = small.tile([128, BPC * NDT * NCH], F32, name="ctxp", tag="ctxp")
        ctxp3 = small.tile([128, NDT * 2], F32, name="ctxp3", tag="ctxp3")

        # scores accumulate col-tiled into one persistent PSUM bank
        # (partitions 0/32/64/96); rest of the bank stays memset-zero.
        # One bank is enough: consecutive chunks' scores are ~30us apart.
        pscb0 = psc.tile([128, SCH], F32, name="pscb0", tag="pscb0")
        nc.vector.memset(pscb0, 0.0)
        ones4 = small.tile([128, 1], BF16, name="ones4", tag="ones4")
        nc.vector.memset(ones4, 0.0)
        for q in range(4):
            nc.vector.memset(ones4[32 * q:32 * q + 1], 1.0)
        # row-of-ones for PE partition-broadcast in the kernel tail
        ones_r = small.tile([1, 128], BF16, name="ones_r", tag="ones_r")
        nc.vector.memset(ones_r, 1.0)
        ones_rf = small.tile([1, 128], F32, name="ones_rf", tag="ones_rf")
        nc.vector.memset(ones_rf, 1.0)

        # --- hproj[h, b] = W1.T @ hidden.T + b_attn ---
        # j 0..3 before the main loop; j 4..7 between chunk 0's halves so
        # chunk 0's first matmuls aren't queued behind the w1 jg1 DMA
        def emit_hproj(j0, j1):
            for j in range(j0, j1):
                ph = pmisc.tile([128, BPC], F32, name=f"ph{j}", tag="pm")
                for k in range(NKT):
                    nc.tensor.matmul(
                        ph,
                        w1_sb[:, j, k, :],
                        hidT_sb[:, k, :],
                        start=(k == 0),
                        stop=(k == NKT - 1),
                    )
                nc.scalar.activation(
                    out=hproj[:, j, :],
                    in_=ph,
                    func=mybir.ActivationFunctionType.Identity,
                    bias=ba_sb[:, j:j + 1],
                    scale=1.0,
                )

        # (hproj emission is deferred into chunk 0's halves below so the PE
        # FIFO reaches chunk 0's matmuls as soon as the first k-pair lands)

        def emit_scores_ctx(b, c, et, soff, sl, dcol, colf, energies, last):
            """scores + exp + ctx partials for s-slice [soff,soff+sl) of
            chunk (b,c). colf(k) -> ctx partial column AP."""
            # scores: 8 col-tiled rank-1 matmuls (4 column groups run
            # concurrently), partial rows at partitions 0/32/64/96,
            # then one ones-matmul sums the rows
            pscb = pscb0
            for j in range(NHT):
                q = j % 4
                # has_written clear is per-element: each column group's
                # row needs its own start (j<4) / stop (j>=4)
                nc.tensor.matmul(
                    pscb[32 * q:32 * q + 1, soff:soff + sl],
                    wv_sb[:, j:j + 1],
                    energies[j],
                    start=(j < 4),
                    stop=(j >= 4),
                    tile_position=(0, 32 * q),
                    skip_group_check=True,
                )
            scp = bcp.tile([128, sl], BF16, name=f"scp{b}_{c}_{soff}",
                           tag="scp")
            nc.vector.tensor_copy(scp, pscb[:, soff:soff + sl])
            ps = pmisc.tile([1, sl], F32, name=f"ps_{b}_{c}_{soff}", tag="pm")
            nc.tensor.matmul(ps, ones4, scp, start=True, stop=True)

            prow = attn_rows[b][0:1, c * SCH + soff:c * SCH + soff + sl]
            nc.scalar.activation(
                out=prow,
                in_=ps,
                func=mybir.ActivationFunctionType.Exp,
                accum_out=denp[0:1, dcol:dcol + 1],
            )

            # broadcast p across partitions, then per-d-tile mul+reduce
            prow_bf = bcp.tile([1, sl], BF16, name=f"pbf{b}_{c}_{soff}",
                               tag="prow_bf")
            nc.scalar.activation(
                out=prow_bf,
                in_=prow,
                func=mybir.ActivationFunctionType.Identity,
            )
            pbc = bcp.tile([128, sl], BF16, name=f"pbc{b}_{c}_{soff}",
                           tag="pbc")
            if last:
                # tail: PE is idle and GpSimd pays a drain between
                # broadcasts — broadcast via ones-matmul instead
                psbc = pmisc.tile([128, sl], F32, name=f"psb{b}_{c}_{soff}",
                                  tag="pm")
                nc.tensor.matmul(psbc, ones_r, prow_bf, start=True, stop=True)
                nc.vector.tensor_copy(pbc, psbc)
            else:
                nc.gpsimd.partition_broadcast(pbc, prow_bf)

            for k in range(NDT):
                prod = scr.tile([128, sl], BF16, name=f"pr{b}_{c}_{soff}_{k}",
                                tag="prod")
                nc.vector.tensor_mul(prod, et[:, k, soff:soff + sl], pbc)
                # last chunk trails the kernel: alternate engines so
                # DVE and ACT drain its reduces concurrently
                if (k % 2 == 0) if last else (k < dve_split):
                    nc.vector.tensor_reduce(
                        colf(k),
                        prod,
                        axis=mybir.AxisListType.X,
                        op=mybir.AluOpType.add,
                    )
                else:
                    prod2 = scr.tile([128, sl], BF16,
                                     name=f"p2{b}_{c}_{soff}_{k}", tag="prod2")
                    nc.scalar.activation(
                        out=prod2,
                        in_=prod,
                        func=mybir.ActivationFunctionType.Identity,
                        accum_out=colf(k),
                    )

        def emit_body(b, c, et, soff, sl, dcol, colf, last):
            energies = []
            for j in range(NHT):
                pp = ppre.tile([128, sl], F32, name=f"pp{b}_{c}_{soff}_{j}",
                               tag="pp")
                for k in range(NDT):
                    nc.tensor.matmul(
                        pp,
                        w2_sb[:, k, j * 128:(j + 1) * 128],
                        et[:, k, soff:soff + sl],
                        start=(k == 0),
                        stop=(k == NDT - 1),
                    )
                en = enp.tile([128, sl], BF16, name=f"en{b}_{c}_{soff}_{j}",
                              tag="en")
                nc.scalar.activation(
                    out=en,
                    in_=pp,
                    func=mybir.ActivationFunctionType.Tanh,
                    bias=hproj[:, j, b:b + 1],
                    scale=1.0,
                )
                energies.append(en)
            emit_scores_ctx(b, c, et, soff, sl, dcol, colf, energies, last)

        # --- main loop ---
        for b in range(BPC):
            for c in range(NCH):
                if b == 0 and c == 0:
                    et = et0
                else:
                    et = encp.tile([128, NDT, SCH], BF16, tag="et")
                    nc.sync.dma_start(out=et, in_=encT[b, c])

                def colf_std(k, b=b, c=c):
                    col = (b * NDT + k) * NCH + c
                    return ctxp[:, col:col + 1]

                if b == 0 and c == 0:
                    # k-outer passes sized so PE consumption (~1.3us/k for
                    # 6 j's) matches the startup DMA arrival rate (~1.4us/k)
                    energies = [None] * NHT
                    for j0, j1 in ((0, 6), (6, 8)):
                        js = range(j0, j1)
                        pps = {j: ppre.tile([128, SCH], F32, name=f"pp0_{j}",
                                            tag="pp") for j in js}
                        for k in range(NDT):
                            for j in js:
                                nc.tensor.matmul(
                                    pps[j],
                                    w2_sb[:, k, j * 128:(j + 1) * 128],
                                    et[:, k, :],
                                    start=(k == 0),
                                    stop=(k == NDT - 1),
                                )
                        if j0 == 0:
                            emit_hproj(0, 8)
                        for j in js:
                            en = enp.tile([128, SCH], BF16, name=f"en00_{j}",
                                          tag="en")
                            nc.scalar.activation(
                                out=en,
                                in_=pps[j],
                                func=mybir.ActivationFunctionType.Tanh,
                                bias=hproj[:, j, b:b + 1],
                                scale=1.0,
                            )
                            energies[j] = en
                    emit_scores_ctx(b, c, et, 0, SCH, b * NCH + c,
                                    colf_std, energies, False)
                elif b == BPC - 1 and c == NCH - 1:
                    # final chunk as two s-halves: the first half's softmax/
                    # ctx pipeline drains under the second half's matmuls
                    for h in range(2):
                        def colf3(k, h=h):
                            return ctxp3[:, k * 2 + h:k * 2 + h + 1]
                        emit_body(b, c, et, h * (SCH // 2), SCH // 2,
                                  BPC * NCH - 1 + h, colf3, True)
                else:
                    emit_body(b, c, et, 0, SCH, b * NCH + c, colf_std, False)

            # --- per-batch finalize, overlaps next batch's chunks ---
            lastb = b == BPC - 1
            dent = outp.tile([1, 1], F32, tag="dent")
            nc.vector.tensor_reduce(
                dent,
                denp[0:1, b * NCH:(b + 1) * NCH + (1 if lastb else 0)],
                axis=mybir.AxisListType.X,
                op=mybir.AluOpType.add,
            )
            rv = outp.tile([1, 1], F32, tag="rv")
            nc.vector.reciprocal(rv, dent)

            attn_f = outp.tile([1, S], F32, tag="attnf")
            nc.vector.tensor_scalar_mul(attn_f, attn_rows[b], rv)
            nc.sync.dma_start(out=attn_out[b:b + 1, :], in_=attn_f)

            ctxr = outp.tile([128, NDT], F32, tag="ctxr")
            if lastb:
                # last batch: c 0..2 from ctxp, split chunk from ctxp3
                r1 = outp.tile([128, NDT], F32, tag="ctxr1")
                nc.vector.tensor_reduce(
                    r1,
                    ctxp.rearrange("p (x c) -> p x c", c=NCH)[
                        :, b * NDT:(b + 1) * NDT, 0:NCH - 1],
                    axis=mybir.AxisListType.X,
                    op=mybir.AluOpType.add,
                )
                r2 = outp.tile([128, NDT], F32, tag="ctxr2")
                nc.vector.tensor_reduce(
                    r2,
                    ctxp3.rearrange("p (x h) -> p x h", h=2),
                    axis=mybir.AxisListType.X,
                    op=mybir.AluOpType.add,
                )
                nc.vector.tensor_add(ctxr, r1, r2)
            else:
                nc.vector.tensor_reduce(
                    ctxr,
                    ctxp.rearrange("p (x c) -> p x c", c=NCH)[
                        :, b * NDT:(b + 1) * NDT, :],
                    axis=mybir.AxisListType.X,
                    op=mybir.AluOpType.add,
                )
            rvb = outp.tile([128, 1], F32, tag="rvb")
            if lastb:
                rvp = pmisc.tile([128, 1], F32, name="rvp", tag="pm")
                nc.tensor.matmul(rvp, ones_rf, rv, start=True, stop=True)
                nc.vector.tensor_copy(rvb, rvp)
            else:
                nc.gpsimd.partition_broadcast(rvb, rv)
            ctxf = outp.tile([128, NDT], F32, tag="ctxf")
            nc.vector.tensor_scalar_mul(ctxf, ctxr, rvb)
            nc.sync.dma_start(out=ctx_out[b], in_=ctxf)

    nc.compile()
    return nc


def _get_nc():
    if "nc" not in _CACHE:
        _CACHE["nc"] = _build()
    return _CACHE["nc"]


def _prep_inputs(hidden, encoder_outputs, W_attn, b_attn, w_v, b_v):
    bf16 = ml_dtypes.bfloat16
    # W1 (H,H) -> (128, NKT, H): partition p holds rows p, p+128, ...
    # W1 (H,H) -> (128, NHT, NKT, 128): [k*128+p, j*128+col] -> [p, j, k, col]
    w1 = np.ascontiguousarray(
        W_attn[:H].reshape(NKT, 128, NHT, 128).transpose(1, 2, 0, 3)).astype(bf16)
    w2 = np.ascontiguousarray(
        W_attn[H:].reshape(NDT, 128, H).transpose(1, 0, 2)).astype(bf16)
    wv_ = np.ascontiguousarray(w_v.reshape(NHT, 128).T).astype(bf16)
    ba_ = np.ascontiguousarray(
        np.asarray(b_attn, dtype=np.float32).reshape(NHT, 128).T)
    enc_bf = encoder_outputs.astype(bf16)  # cast first (fast)
    in_maps = []
    for core in range(NCORES):
        sl = slice(core * BPC, (core + 1) * BPC)
        # (b, s, d) -> (b, c, p, k, s): s = c*SCH + s', d = k*128 + p
        encT = np.ascontiguousarray(
            enc_bf[sl].reshape(BPC, NCH, SCH, NDT, 128).transpose(0, 1, 4, 3, 2))
        hidT = np.ascontiguousarray(
            hidden[sl].T.reshape(NKT, 128, BPC).transpose(1, 0, 2)).astype(bf16)
        in_maps.append(
            {
                "enct": encT,
                "w2": w2,
                "w1": w1,
                "hidt": hidT,
                "wv": wv_,
                "ba": ba_,
            }
        )
    return in_maps


def kernel(hidden, encoder_outputs, W_attn, b_attn, w_v, b_v, _trace=False):
    nc = _get_nc()
    in_maps = _prep_inputs(hidden, encoder_outputs, W_attn, b_attn, w_v, b_v)
    res = run_bass_kernel_spmd(
        nc, in_maps, core_ids=list(range(NCORES)), trace=_trace
    )
    # ctx arrives as (BPC, 128, NDT); d = k*128 + p
    context = np.concatenate(
        [r["ctx"].transpose(0, 2, 1).reshape(BPC, D) for r in res.results],
        axis=0)
    attn = np.concatenate([r["attn"] for r in res.results], axis=0)
    if _trace:
        _CACHE["last_results"] = res
    return context, attn


# revision 48
# speedup vs baseline: 1.0052x; 1.0032x over previous
"""Trainium2 Bass kernel for the additive-attention layer.

Math (per batch b):
    pre[s, h]   = enc[b] @ W2 + hidden[b] @ W1 + b_attn      (W1=W_attn[:H], W2=W_attn[H:])
    energy      = tanh(pre)
    scores[s]   = energy @ w_v (+ b_v, irrelevant: softmax is shift-invariant)
    attn        = softmax(scores)
    context     = attn @ enc[b]

Distribution: data-parallel over batch, 4 batches per core, no collectives.

Device dataflow per (batch, s-chunk of 512):
  - encT tiles (d on partitions, s free) arrive via DMA; the host pre-tiles
    to (b, c, p, k, s) so each chunk DMA is 16KB-contiguous per partition.
  - TensorE: psum[h128, s512] += W2[d128, h128].T @ encT[d128, s512]  (16 d-tiles)
  - ScalarE: energy = tanh(psum + hproj[b]) -> SBUF bf16   (hproj = W1.T@hidden + b_attn)
  - TensorE: scores_psum[1, s512] += w_v[h128, 1].T @ energy  (8 h-tiles)
  - ScalarE: p = exp(scores) -> attn row, accum_out = chunk denominator
  - GpSimd: broadcast p across 128 partitions
  - ctx partials: prod = encT * p on DVE; free-dim reduce split between DVE
    tensor_reduce (k < DVE_SPLIT) and ACT Identity accum (rest) so neither
    engine's FIFO backs up and gates PSUM-bank release for the next chunk.
Finalize is per-batch, emitted right after that batch's last chunk so it
overlaps the next batch's main loop; only the last batch's ~3us trails.
"""

import numpy as np
import ml_dtypes
from contextlib import ExitStack

import concourse.bacc as bacc
import concourse.bass as bass
import concourse.tile as tile
import concourse.mybir as mybir
from concourse.bass_utils import run_bass_kernel_spmd

B, S, H = 32, 2048, 1024
D = 2 * H                     # encoder feature dim
NCORES = 8
BPC = B // NCORES             # batches per core
SCH = 512                     # s-chunk (one PSUM bank of fp32)
NCH = S // SCH
NDT = D // 128                # d-tiles (contraction tiles for main matmul)
NHT = H // 128                # h-tiles
NKT = H // 128                # d-tiles for the W1 projection
KG = 4                        # k-tiles per startup sub-DMA group

BF16 = mybir.dt.bfloat16
F32 = mybir.dt.float32

_CACHE = {}


def _build(encp_bufs=4, enp_bufs=16, ppre_bufs=6, psc_bufs=1, scr_bufs=6,
           dve_split=6):
    nc = bacc.Bacc("TRN2", target_bir_lowering=False, debug=False)

    # Host pre-tiles everything into partition-major layouts so every
    # device DMA is one contiguous run per partition (big descriptors).
    encT = nc.dram_tensor("enct", (BPC, NCH, 128, NDT, SCH), BF16,
                          kind="ExternalInput").ap()
    w2 = nc.dram_tensor("w2", (128, NDT, H), BF16, kind="ExternalInput").ap()
    # w1 j-major so hproj j-groups can start after 1MB of DMA
    w1 = nc.dram_tensor("w1", (128, NHT, NKT, 128), BF16, kind="ExternalInput").ap()
    hidT = nc.dram_tensor("hidt", (128, NKT, BPC), BF16, kind="ExternalInput").ap()
    wv = nc.dram_tensor("wv", (128, NHT), BF16, kind="ExternalInput").ap()
    ba = nc.dram_tensor("ba", (128, NHT), F32, kind="ExternalInput").ap()
    # ctx in partition-major (b, p, k); host untransposes
    ctx_out = nc.dram_tensor("ctx", (BPC, 128, NDT), F32, kind="ExternalOutput").ap()
    attn_out = nc.dram_tensor("attn", (BPC, S), F32, kind="ExternalOutput").ap()

    with tile.TileContext(nc) as tc, ExitStack() as ctx:
        weights = ctx.enter_context(tc.tile_pool(name="weights", bufs=1))
        encp = ctx.enter_context(tc.tile_pool(name="encp", bufs=encp_bufs))
        enp = ctx.enter_context(tc.tile_pool(name="enp", bufs=enp_bufs))
        small = ctx.enter_context(tc.tile_pool(name="small", bufs=1))
        bcp = ctx.enter_context(tc.tile_pool(name="bcp", bufs=4))
        scr = ctx.enter_context(tc.tile_pool(name="scr", bufs=scr_bufs))
        outp = ctx.enter_context(tc.tile_pool(name="outp", bufs=2))
        ppre = ctx.enter_context(tc.tile_pool(name="ppre", bufs=ppre_bufs, space="PSUM"))
        psc = ctx.enter_context(tc.tile_pool(name="psc", bufs=psc_bufs, space="PSUM"))
        pmisc = ctx.enter_context(tc.tile_pool(name="pmisc", bufs=1, space="PSUM"))

        # --- resident small weights first (hproj path warms PE early) ---
        hidT_sb = small.tile([128, NKT, BPC], BF16)
        nc.sync.dma_start(out=hidT_sb, in_=hidT)
        wv_sb = small.tile([128, NHT], BF16)
        nc.sync.dma_start(out=wv_sb, in_=wv)
        ba_sb = small.tile([128, NHT], F32)
        nc.sync.dma_start(out=ba_sb, in_=ba)
        # w1 j-halves interleaved with w2/enc k-groups: hproj can start
        # after ~1MB; chunk 0's first matmuls after ~2.5MB instead of 8MB
        w1_sb = weights.tile([128, NHT, NKT, 128], BF16)
        w2_sb = weights.tile([128, NDT, H], BF16)
        et0 = encp.tile([128, NDT, SCH], BF16, name="et0", tag="et")
        # single-k first pieces so chunk 0's first matmul starts after ~0.6MB
        for k in range(2):
            nc.sync.dma_start(out=w2_sb[:, k:k + 1, :], in_=w2[:, k:k + 1, :])
            nc.sync.dma_start(out=et0[:, k:k + 1, :],
                              in_=encT[0, 0, :, k:k + 1, :])
        for kk in range(1, NDT // 2):
            nc.sync.dma_start(out=w2_sb[:, kk * 2:kk * 2 + 2, :],
                              in_=w2[:, kk * 2:kk * 2 + 2, :])
            nc.sync.dma_start(out=et0[:, kk * 2:kk * 2 + 2, :],
                              in_=encT[0, 0, :, kk * 2:kk * 2 + 2, :])
            # hproj runs after chunk 0's first pass (~24us in), so w1 rides
            # at the back of the critical k-stream
            if kk == 5:
                nc.sync.dma_start(out=w1_sb[:, 0:4], in_=w1[:, 0:4])
            if kk == 7:
                nc.sync.dma_start(out=w1_sb[:, 4:8], in_=w1[:, 4:8])

        # --- persistent accumulators ---
        hproj = small.tile([128, NHT, BPC], F32, name="hproj", tag="hproj")
        attn_rows = [
            small.tile([1, S], F32, name=f"attnrow{b}", tag=f"attnrow{b}")
            for b in range(BPC)
        ]
        # +1 col: the last chunk is split into two s-halves (cols 15, 16)
        denp = small.tile([1, BPC * NCH + 1], F32, name="denp", tag="denp")
        # context partials: column layout (b, k, c); the split last chunk
        # gets its own (k, h) block
        ctxp = small.tile([128, BPC * NDT * NCH], F32, name="ctxp", tag="ctxp")
        ctxp3 = small.tile([128, NDT * 2], F32, name="ctxp3", tag="ctxp3")

        # scores accumulate col-tiled into one persistent PSUM bank
        # (partitions 0/32/64/96); rest of the bank stays memset-zero.
        # One bank is enough: consecutive chunks' scores are ~30us apart.
        pscb0 = psc.tile([128, SCH], F32, name="pscb0", tag="pscb0")
        nc.vector.memset(pscb0, 0.0)
        ones4 = small.tile([128, 1], BF16, name="ones4", tag="ones4")
        nc.vector.memset(ones4, 0.0)
        for q in range(4):
            nc.vector.memset(ones4[32 * q:32 * q + 1], 1.0)
        # row-of-ones for PE partition-broadcast in the kernel tail
        ones_r = small.tile([1, 128], BF16, name="ones_r", tag="ones_r")
        nc.vector.memset(ones_r, 1.0)
        ones_rf = small.tile([1, 128], F32, name="ones_rf", tag="ones_rf")
        nc.vector.memset(ones_rf, 1.0)

        # --- hproj[h, b] = W1.T @ hidden.T + b_attn ---
        # j 0..3 before the main loop; j 4..7 between chunk 0's halves so
        # chunk 0's first matmuls aren't queued behind the w1 jg1 DMA
        def emit_hproj(j0, j1):
            for j in range(j0, j1):
                ph = pmisc.tile([128, BPC], F32, name=f"ph{j}", tag="pm")
                for k in range(NKT):
                    nc.tensor.matmul(
                        ph,
                        w1_sb[:, j, k, :],
                        hidT_sb[:, k, :],
                        start=(k == 0),
                        stop=(k == NKT - 1),
                    )
                nc.scalar.activation(
                    out=hproj[:, j, :],
                    in_=ph,
                    func=mybir.ActivationFunctionType.Identity,
                    bias=ba_sb[:, j:j + 1],
                    scale=1.0,
                )

        # (hproj emission is deferred into chunk 0's halves below so the PE
        # FIFO reaches chunk 0's matmuls as soon as the first k-pair lands)

        def emit_scores_ctx(b, c, et, soff, sl, dcol, colf, energies, last):
            """scores + exp + ctx partials for s-slice [soff,soff+sl) of
            chunk (b,c). colf(k) -> ctx partial column AP."""
            # scores: 8 col-tiled rank-1 matmuls (4 column groups run
            # concurrently), partial rows at partitions 0/32/64/96,
            # then one ones-matmul sums the rows
            pscb = pscb0
            for j in range(NHT):
                q = j % 4
                # has_written clear is per-element: each column group's
                # row needs its own start (j<4) / stop (j>=4)
                nc.tensor.matmul(
                    pscb[32 * q:32 * q + 1, soff:soff + sl],
                    wv_sb[:, j:j + 1],
                    energies[j],
                    start=(j < 4),
                    stop=(j >= 4),
                    tile_position=(0, 32 * q),
                    skip_group_check=True,
                )
            scp = bcp.tile([128, sl], BF16, name=f"scp{b}_{c}_{soff}",
                           tag="scp")
            nc.vector.tensor_copy(scp, pscb[:, soff:soff + sl])
            ps = pmisc.tile([1, sl], F32, name=f"ps_{b}_{c}_{soff}", tag="pm")
            nc.tensor.matmul(ps, ones4, scp, start=True, stop=True)

            prow = attn_rows[b][0:1, c * SCH + soff:c * SCH + soff + sl]
            nc.scalar.activation(
                out=prow,
                in_=ps,
                func=mybir.ActivationFunctionType.Exp,
                accum_out=denp[0:1, dcol:dcol + 1],
            )

            # broadcast p across partitions, then per-d-tile mul+reduce
            prow_bf = bcp.tile([1, sl], BF16, name=f"pbf{b}_{c}_{soff}",
                               tag="prow_bf")
            nc.scalar.activation(
                out=prow_bf,
                in_=prow,
                func=mybir.ActivationFunctionType.Identity,
            )
            pbc = bcp.tile([128, sl], BF16, name=f"pbc{b}_{c}_{soff}",
                           tag="pbc")
            if last:
                # tail: PE is idle and GpSimd pays a drain between
                # broadcasts — broadcast via ones-matmul instead
                psbc = pmisc.tile([128, sl], F32, name=f"psb{b}_{c}_{soff}",
                                  tag="pm")
                nc.tensor.matmul(psbc, ones_r, prow_bf, start=True, stop=True)
                nc.vector.tensor_copy(pbc, psbc)
            else:
                nc.gpsimd.partition_broadcast(pbc, prow_bf)

            for k in range(NDT):
                prod = scr.tile([128, sl], BF16, name=f"pr{b}_{c}_{soff}_{k}",
                                tag="prod")
                nc.vector.tensor_mul(prod, et[:, k, soff:soff + sl], pbc)
                # last chunk trails the kernel: alternate engines so
                # DVE and ACT drain its reduces concurrently
                if (k % 2 == 0) if last else (k < dve_split):
                    nc.vector.tensor_reduce(
                        colf(k),
                        prod,
                        axis=mybir.AxisListType.X,
                        op=mybir.AluOpType.add,
                    )
                else:
                    prod2 = scr.tile([128, sl], BF16,
                                     name=f"p2{b}_{c}_{soff}_{k}", tag="prod2")
                    nc.scalar.activation(
                        out=prod2,
                        in_=prod,
                        func=mybir.ActivationFunctionType.Identity,
                        accum_out=colf(k),
                    )

        def emit_body(b, c, et, soff, sl, dcol, colf, last):
            energies = []
            for j in range(NHT):
                pp = ppre.tile([128, sl], F32, name=f"pp{b}_{c}_{soff}_{j}",
                               tag="pp")
                for k in range(NDT):
                    nc.tensor.matmul(
                        pp,
                        w2_sb[:, k, j * 128:(j + 1) * 128],
                        et[:, k, soff:soff + sl],
                        start=(k == 0),
                        stop=(k == NDT - 1),
                    )
                en = enp.tile([128, sl], BF16, name=f"en{b}_{c}_{soff}_{j}",
                              tag="en")
                nc.scalar.activation(
                    out=en,
                    in_=pp,
                    func=mybir.ActivationFunctionType.Tanh,
                    bias=hproj[:, j, b:b + 1],
                    scale=1.0,
                )
                energies.append(en)
            emit_scores_ctx(b, c, et, soff, sl, dcol, colf, energies, last)

        # --- main loop ---
        for b in range(BPC):
            for c in range(NCH):
                if b == 0 and c == 0:
                    et = et0
                else:
                    et = encp.tile([128, NDT, SCH], BF16, tag="et")
                    nc.sync.dma_start(out=et, in_=encT[b, c])

                def colf_std(k, b=b, c=c):
                    col = (b * NDT + k) * NCH + c
                    return ctxp[:, col:col + 1]

                if b == 0 and c == 0:
                    # k-outer passes sized so PE consumption (~1.3us/k for
                    # 6 j's) matches the startup DMA arrival rate (~1.4us/k)
                    energies = [None] * NHT
                    for j0, j1 in ((0, 6), (6, 8)):
                        js = range(j0, j1)
                        pps = {j: ppre.tile([128, SCH], F32, name=f"pp0_{j}",
                                            tag="pp") for j in js}
                        for k in range(NDT):
                            for j in js:
                                nc.tensor.matmul(
                                    pps[j],
                                    w2_sb[:, k, j * 128:(j + 1) * 128],
                                    et[:, k, :],
                                    start=(k == 0),
                                    stop=(k == NDT - 1),
                                )
                        if j0 == 0:
                            emit_hproj(0, 8)
                        for j in js:
                            en = enp.tile([128, SCH], BF16, name=f"en00_{j}",
                                          tag="en")
                            nc.scalar.activation(
                                out=en,
                                in_=pps[j],
                                func=mybir.ActivationFunctionType.Tanh,
                                bias=hproj[:, j, b:b + 1],
                                scale=1.0,
                            )
                            energies[j] = en
                    emit_scores_ctx(b, c, et, 0, SCH, b * NCH + c,
                                    colf_std, energies, False)
                elif b == BPC - 1 and c == NCH - 1:
                    # final chunk as two s-halves: the first half's softmax/
                    # ctx pipeline drains under the second half's matmuls
                    for h in range(2):
                        def colf3(k, h=h):
                            return ctxp3[:, k * 2 + h:k * 2 + h + 1]
                        emit_body(b, c, et, h * (SCH // 2), SCH // 2,
                                  BPC * NCH - 1 + h, colf3, True)
                else:
                    emit_body(b, c, et, 0, SCH, b * NCH + c, colf_std, False)

            # --- per-batch finalize, overlaps next batch's chunks ---
            lastb = b == BPC - 1
            dent = outp.tile([1, 1], F32, tag="dent")
            nc.vector.tensor_reduce(
                dent,
                denp[0:1, b * NCH:(b + 1) * NCH + (1 if lastb else 0)],
                axis=mybir.AxisListType.X,
                op=mybir.AluOpType.add,
            )
            rv = outp.tile([1, 1], F32, tag="rv")
            nc.vector.reciprocal(rv, dent)

            attn_f = outp.tile([1, S], F32, tag="attnf")
            nc.vector.tensor_scalar_mul(attn_f, attn_rows[b], rv)
            nc.sync.dma_start(out=attn_out[b:b + 1, :], in_=attn_f)

            ctxr = outp.tile([128, NDT], F32, tag="ctxr")
            if lastb:
                # last batch: c 0..2 from ctxp, split chunk from ctxp3
                r1 = outp.tile([128, NDT], F32, tag="ctxr1")
                nc.vector.tensor_reduce(
                    r1,
                    ctxp.rearrange("p (x c) -> p x c", c=NCH)[
                        :, b * NDT:(b + 1) * NDT, 0:NCH - 1],
                    axis=mybir.AxisListType.X,
                    op=mybir.AluOpType.add,
                )
                r2 = outp.tile([128, NDT], F32, tag="ctxr2")
                nc.vector.tensor_reduce(
                    r2,
                    ctxp3.rearrange("p (x h) -> p x h", h=2),
                    axis=mybir.AxisListType.X,
                    op=mybir.AluOpType.add,
                )
                nc.vector.tensor_add(ctxr, r1, r2)
            else:
                nc.vector.tensor_reduce(
                    ctxr,
                    ctxp.rearrange("p (x c) -> p x c", c=NCH)[
                        :, b * NDT:(b + 1) * NDT, :],
                    axis=mybir.AxisListType.X,
                    op=mybir.AluOpType.add,
                )
            rvb = outp.tile([128, 1], F32, tag="rvb")
            if lastb:
                rvp = pmisc.tile([128, 1], F32, name="rvp", tag="pm")
                nc.tensor.matmul(rvp, ones_rf, rv, start=True, stop=True)
                nc.vector.tensor_copy(rvb, rvp)
            else:
                nc.gpsimd.partition_broadcast(rvb, rv)
            ctxf = outp.tile([128, NDT], F32, tag="ctxf")
            nc.vector.tensor_scalar_mul(ctxf, ctxr, rvb)
            nc.sync.dma_start(out=ctx_out[b], in_=ctxf)

    nc.compile()
    return nc


def _get_nc():
    if "nc" not in _CACHE:
        _CACHE["nc"] = _build()
    return _CACHE["nc"]


def _prep_inputs(hidden, encoder_outputs, W_attn, b_attn, w_v, b_v):
    bf16 = ml_dtypes.bfloat16
    # W1 (H,H) -> (128, NKT, H): partition p holds rows p, p+128, ...
    # W1 (H,H) -> (128, NHT, NKT, 128): [k*128+p, j*128+col] -> [p, j, k, col]
    w1 = np.ascontiguousarray(
        W_attn[:H].reshape(NKT, 128, NHT, 128).transpose(1, 2, 0, 3)).astype(bf16)
    w2 = np.ascontiguousarray(
        W_attn[H:].reshape(NDT, 128, H).transpose(1, 0, 2)).astype(bf16)
    wv_ = np.ascontiguousarray(w_v.reshape(NHT, 128).T).astype(bf16)
    ba_ = np.ascontiguousarray(
        np.asarray(b_attn, dtype=np.float32).reshape(NHT, 128).T)
    enc_bf = encoder_outputs.astype(bf16)  # cast first (fast)
    in_maps = []
    for core in range(NCORES):
        sl = slice(core * BPC, (core + 1) * BPC)
        # (b, s, d) -> (b, c, p, k, s): s = c*SCH + s', d = k*128 + p
        encT = np.ascontiguousarray(
            enc_bf[sl].reshape(BPC, NCH, SCH, NDT, 128).transpose(0, 1, 4, 3, 2))
        hidT = np.ascontiguousarray(
            hidden[sl].T.reshape(NKT, 128, BPC).transpose(1, 0, 2)).astype(bf16)
        in_maps.append(
            {
                "enct": encT,
                "w2": w2,
                "w1": w1,
                "hidt": hidT,
                "wv": wv_,
                "ba": ba_,
            }
        )
    return in_maps


def kernel(hidden, encoder_outputs, W_attn, b_attn, w_v, b_v, _trace=False):
    nc = _get_nc()
    in_maps = _prep_inputs(hidden, encoder_outputs, W_attn, b_attn, w_v, b_v)
    res = run_bass_kernel_spmd(
        nc, in_maps, core_ids=list(range(NCORES)), trace=_trace
    )
    # ctx arrives as (BPC, 128, NDT); d = k*128 + p
    context = np.concatenate(
        [r["ctx"].transpose(0, 2, 1).reshape(BPC, D) for r in res.results],
        axis=0)
    attn = np.concatenate([r["attn"] for r in res.results], axis=0)
    if _trace:
        _CACHE["last_results"] = res
    return context, attn


# revision 49
# speedup vs baseline: 1.0053x; 1.0001x over previous
"""Trainium2 Bass kernel for the additive-attention layer.

Math (per batch b):
    pre[s, h]   = enc[b] @ W2 + hidden[b] @ W1 + b_attn      (W1=W_attn[:H], W2=W_attn[H:])
    energy      = tanh(pre)
    scores[s]   = energy @ w_v (+ b_v, irrelevant: softmax is shift-invariant)
    attn        = softmax(scores)
    context     = attn @ enc[b]

Distribution: data-parallel over batch, 4 batches per core, no collectives.

Device dataflow per (batch, s-chunk of 512):
  - encT tiles (d on partitions, s free) arrive via DMA; the host pre-tiles
    to (b, c, p, k, s) so each chunk DMA is 16KB-contiguous per partition.
  - TensorE: psum[h128, s512] += W2[d128, h128].T @ encT[d128, s512]  (16 d-tiles)
  - ScalarE: energy = tanh(psum + hproj[b]) -> SBUF bf16   (hproj = W1.T@hidden + b_attn)
  - TensorE: scores_psum[1, s512] += w_v[h128, 1].T @ energy  (8 h-tiles)
  - ScalarE: p = exp(scores) -> attn row, accum_out = chunk denominator
  - GpSimd: broadcast p across 128 partitions
  - ctx partials: prod = encT * p on DVE; free-dim reduce split between DVE
    tensor_reduce (k < DVE_SPLIT) and ACT Identity accum (rest) so neither
    engine's FIFO backs up and gates PSUM-bank release for the next chunk.
Finalize is per-batch, emitted right after that batch's last chunk so it
overlaps the next batch's main loop; only the last batch's ~3us trails.
"""

import numpy as np
import ml_dtypes
from contextlib import ExitStack

import concourse.bacc as bacc
import concourse.bass as bass
import concourse.tile as tile
import concourse.mybir as mybir
from concourse.bass_utils import run_bass_kernel_spmd

B, S, H = 32, 2048, 1024
D = 2 * H                     # encoder feature dim
NCORES = 8
BPC = B // NCORES             # batches per core
SCH = 512                     # s-chunk (one PSUM bank of fp32)
NCH = S // SCH
NDT = D // 128                # d-tiles (contraction tiles for main matmul)
NHT = H // 128                # h-tiles
NKT = H // 128                # d-tiles for the W1 projection
KG = 4                        # k-tiles per startup sub-DMA group

BF16 = mybir.dt.bfloat16
F32 = mybir.dt.float32

_CACHE = {}


def _build(encp_bufs=4, enp_bufs=12, ppre_bufs=6, psc_bufs=1, scr_bufs=6,
           dve_split=6):
    nc = bacc.Bacc("TRN2", target_bir_lowering=False, debug=False)

    # Host pre-tiles everything into partition-major layouts so every
    # device DMA is one contiguous run per partition (big descriptors).
    encT = nc.dram_tensor("enct", (BPC, NCH, 128, NDT, SCH), BF16,
                          kind="ExternalInput").ap()
    w2 = nc.dram_tensor("w2", (128, NDT, H), BF16, kind="ExternalInput").ap()
    # w1 j-major so hproj j-groups can start after 1MB of DMA
    w1 = nc.dram_tensor("w1", (128, NHT, NKT, 128), BF16, kind="ExternalInput").ap()
    hidT = nc.dram_tensor("hidt", (128, NKT, BPC), BF16, kind="ExternalInput").ap()
    wv = nc.dram_tensor("wv", (128, NHT), BF16, kind="ExternalInput").ap()
    ba = nc.dram_tensor("ba", (128, NHT), F32, kind="ExternalInput").ap()
    # ctx in partition-major (b, p, k); host untransposes
    ctx_out = nc.dram_tensor("ctx", (BPC, 128, NDT), F32, kind="ExternalOutput").ap()
    attn_out = nc.dram_tensor("attn", (BPC, S), F32, kind="ExternalOutput").ap()

    with tile.TileContext(nc) as tc, ExitStack() as ctx:
        weights = ctx.enter_context(tc.tile_pool(name="weights", bufs=1))
        encp = ctx.enter_context(tc.tile_pool(name="encp", bufs=encp_bufs))
        enp = ctx.enter_context(tc.tile_pool(name="enp", bufs=enp_bufs))
        small = ctx.enter_context(tc.tile_pool(name="small", bufs=1))
        bcp = ctx.enter_context(tc.tile_pool(name="bcp", bufs=4))
        scr = ctx.enter_context(tc.tile_pool(name="scr", bufs=scr_bufs))
        outp = ctx.enter_context(tc.tile_pool(name="outp", bufs=2))
        ppre = ctx.enter_context(tc.tile_pool(name="ppre", bufs=ppre_bufs, space="PSUM"))
        psc = ctx.enter_context(tc.tile_pool(name="psc", bufs=psc_bufs, space="PSUM"))
        pmisc = ctx.enter_context(tc.tile_pool(name="pmisc", bufs=1, space="PSUM"))

        # --- resident small weights first (hproj path warms PE early) ---
        hidT_sb = small.tile([128, NKT, BPC], BF16)
        nc.sync.dma_start(out=hidT_sb, in_=hidT)
        wv_sb = small.tile([128, NHT], BF16)
        nc.sync.dma_start(out=wv_sb, in_=wv)
        ba_sb = small.tile([128, NHT], F32)
        nc.sync.dma_start(out=ba_sb, in_=ba)
        # w1 j-halves interleaved with w2/enc k-groups: hproj can start
        # after ~1MB; chunk 0's first matmuls after ~2.5MB instead of 8MB
        w1_sb = weights.tile([128, NHT, NKT, 128], BF16)
        w2_sb = weights.tile([128, NDT, H], BF16)
        et0 = encp.tile([128, NDT, SCH], BF16, name="et0", tag="et")
        # single-k first pieces so chunk 0's first matmul starts after ~0.6MB
        for k in range(2):
            nc.sync.dma_start(out=w2_sb[:, k:k + 1, :], in_=w2[:, k:k + 1, :])
            nc.sync.dma_start(out=et0[:, k:k + 1, :],
                              in_=encT[0, 0, :, k:k + 1, :])
        for kk in range(1, NDT // 2):
            nc.sync.dma_start(out=w2_sb[:, kk * 2:kk * 2 + 2, :],
                              in_=w2[:, kk * 2:kk * 2 + 2, :])
            nc.sync.dma_start(out=et0[:, kk * 2:kk * 2 + 2, :],
                              in_=encT[0, 0, :, kk * 2:kk * 2 + 2, :])
            # hproj runs after chunk 0's first pass (~24us in), so w1 rides
            # at the back of the critical k-stream
            if kk == 5:
                nc.sync.dma_start(out=w1_sb[:, 0:4], in_=w1[:, 0:4])
            if kk == 7:
                nc.sync.dma_start(out=w1_sb[:, 4:8], in_=w1[:, 4:8])

        # --- persistent accumulators ---
        hproj = small.tile([128, NHT, BPC], F32, name="hproj", tag="hproj")
        attn_rows = [
            small.tile([1, S], F32, name=f"attnrow{b}", tag=f"attnrow{b}")
            for b in range(BPC)
        ]
        # +1 col: the last chunk is split into two s-halves (cols 15, 16)
        denp = small.tile([1, BPC * NCH + 1], F32, name="denp", tag="denp")
        # context partials: column layout (b, k, c); the split last chunk
        # gets its own (k, h) block
        ctxp = small.tile([128, BPC * NDT * NCH], F32, name="ctxp", tag="ctxp")
        ctxp3 = small.tile([128, NDT * 2], F32, name="ctxp3", tag="ctxp3")

        # scores accumulate col-tiled into one persistent PSUM bank
        # (partitions 0/32/64/96); rest of the bank stays memset-zero.
        # One bank is enough: consecutive chunks' scores are ~30us apart.
        pscb0 = psc.tile([128, SCH], F32, name="pscb0", tag="pscb0")
        nc.vector.memset(pscb0, 0.0)
        ones4 = small.tile([128, 1], BF16, name="ones4", tag="ones4")
        nc.vector.memset(ones4, 0.0)
        for q in range(4):
            nc.vector.memset(ones4[32 * q:32 * q + 1], 1.0)
        # row-of-ones for PE partition-broadcast in the kernel tail
        ones_r = small.tile([1, 128], BF16, name="ones_r", tag="ones_r")
        nc.vector.memset(ones_r, 1.0)
        ones_rf = small.tile([1, 128], F32, name="ones_rf", tag="ones_rf")
        nc.vector.memset(ones_rf, 1.0)

        # --- hproj[h, b] = W1.T @ hidden.T + b_attn ---
        # j 0..3 before the main loop; j 4..7 between chunk 0's halves so
        # chunk 0's first matmuls aren't queued behind the w1 jg1 DMA
        def emit_hproj(j0, j1):
            for j in range(j0, j1):
                ph = pmisc.tile([128, BPC], F32, name=f"ph{j}", tag="pm")
                for k in range(NKT):
                    nc.tensor.matmul(
                        ph,
                        w1_sb[:, j, k, :],
                        hidT_sb[:, k, :],
                        start=(k == 0),
                        stop=(k == NKT - 1),
                    )
                nc.scalar.activation(
                    out=hproj[:, j, :],
                    in_=ph,
                    func=mybir.ActivationFunctionType.Identity,
                    bias=ba_sb[:, j:j + 1],
                    scale=1.0,
                )

        # (hproj emission is deferred into chunk 0's halves below so the PE
        # FIFO reaches chunk 0's matmuls as soon as the first k-pair lands)

        def emit_scores_ctx(b, c, et, soff, sl, dcol, colf, energies, last):
            """scores + exp + ctx partials for s-slice [soff,soff+sl) of
            chunk (b,c). colf(k) -> ctx partial column AP."""
            # scores: 8 col-tiled rank-1 matmuls (4 column groups run
            # concurrently), partial rows at partitions 0/32/64/96,
            # then one ones-matmul sums the rows
            pscb = pscb0
            for j in range(NHT):
                q = j % 4
                # has_written clear is per-element: each column group's
                # row needs its own start (j<4) / stop (j>=4)
                nc.tensor.matmul(
                    pscb[32 * q:32 * q + 1, soff:soff + sl],
                    wv_sb[:, j:j + 1],
                    energies[j],
                    start=(j < 4),
                    stop=(j >= 4),
                    tile_position=(0, 32 * q),
                    skip_group_check=True,
                )
            scp = bcp.tile([128, sl], BF16, name=f"scp{b}_{c}_{soff}",
                           tag="scp")
            nc.vector.tensor_copy(scp, pscb[:, soff:soff + sl])
            ps = pmisc.tile([1, sl], F32, name=f"ps_{b}_{c}_{soff}", tag="pm")
            nc.tensor.matmul(ps, ones4, scp, start=True, stop=True)

            prow = attn_rows[b][0:1, c * SCH + soff:c * SCH + soff + sl]
            nc.scalar.activation(
                out=prow,
                in_=ps,
                func=mybir.ActivationFunctionType.Exp,
                accum_out=denp[0:1, dcol:dcol + 1],
            )

            # broadcast p across partitions, then per-d-tile mul+reduce
            prow_bf = bcp.tile([1, sl], BF16, name=f"pbf{b}_{c}_{soff}",
                               tag="prow_bf")
            nc.scalar.activation(
                out=prow_bf,
                in_=prow,
                func=mybir.ActivationFunctionType.Identity,
            )
            pbc = bcp.tile([128, sl], BF16, name=f"pbc{b}_{c}_{soff}",
                           tag="pbc")
            if last:
                # tail: PE is idle and GpSimd pays a drain between
                # broadcasts — broadcast via ones-matmul instead
                psbc = pmisc.tile([128, sl], F32, name=f"psb{b}_{c}_{soff}",
                                  tag="pm")
                nc.tensor.matmul(psbc, ones_r, prow_bf, start=True, stop=True)
                nc.vector.tensor_copy(pbc, psbc)
            else:
                nc.gpsimd.partition_broadcast(pbc, prow_bf)

            for k in range(NDT):
                prod = scr.tile([128, sl], BF16, name=f"pr{b}_{c}_{soff}_{k}",
                                tag="prod")
                nc.vector.tensor_mul(prod, et[:, k, soff:soff + sl], pbc)
                # last chunk trails the kernel: alternate engines so
                # DVE and ACT drain its reduces concurrently
                if (k % 2 == 0) if last else (k < dve_split):
                    nc.vector.tensor_reduce(
                        colf(k),
                        prod,
                        axis=mybir.AxisListType.X,
                        op=mybir.AluOpType.add,
                    )
                else:
                    prod2 = scr.tile([128, sl], BF16,
                                     name=f"p2{b}_{c}_{soff}_{k}", tag="prod2")
                    nc.scalar.activation(
                        out=prod2,
                        in_=prod,
                        func=mybir.ActivationFunctionType.Identity,
                        accum_out=colf(k),
                    )

        def emit_body(b, c, et, soff, sl, dcol, colf, last):
            energies = []
            for j in range(NHT):
                pp = ppre.tile([128, sl], F32, name=f"pp{b}_{c}_{soff}_{j}",
                               tag="pp")
                for k in range(NDT):
                    nc.tensor.matmul(
                        pp,
                        w2_sb[:, k, j * 128:(j + 1) * 128],
                        et[:, k, soff:soff + sl],
                        start=(k == 0),
                        stop=(k == NDT - 1),
                    )
                en = enp.tile([128, sl], BF16, name=f"en{b}_{c}_{soff}_{j}",
                              tag="en")
                nc.scalar.activation(
                    out=en,
                    in_=pp,
                    func=mybir.ActivationFunctionType.Tanh,
                    bias=hproj[:, j, b:b + 1],
                    scale=1.0,
                )
                energies.append(en)
            emit_scores_ctx(b, c, et, soff, sl, dcol, colf, energies, last)

        # --- main loop ---
        for b in range(BPC):
            for c in range(NCH):
                if b == 0 and c == 0:
                    et = et0
                else:
                    et = encp.tile([128, NDT, SCH], BF16, tag="et")
                    nc.sync.dma_start(out=et, in_=encT[b, c])

                def colf_std(k, b=b, c=c):
                    col = (b * NDT + k) * NCH + c
                    return ctxp[:, col:col + 1]

                if b == 0 and c == 0:
                    # k-outer passes sized so PE consumption (~1.3us/k for
                    # 6 j's) matches the startup DMA arrival rate (~1.4us/k)
                    energies = [None] * NHT
                    for j0, j1 in ((0, 6), (6, 8)):
                        js = range(j0, j1)
                        pps = {j: ppre.tile([128, SCH], F32, name=f"pp0_{j}",
                                            tag="pp") for j in js}
                        for k in range(NDT):
                            for j in js:
                                nc.tensor.matmul(
                                    pps[j],
                                    w2_sb[:, k, j * 128:(j + 1) * 128],
                                    et[:, k, :],
                                    start=(k == 0),
                                    stop=(k == NDT - 1),
                                )
                        if j0 == 0:
                            emit_hproj(0, 8)
                        for j in js:
                            en = enp.tile([128, SCH], BF16, name=f"en00_{j}",
                                          tag="en")
                            nc.scalar.activation(
                                out=en,
                                in_=pps[j],
                                func=mybir.ActivationFunctionType.Tanh,
                                bias=hproj[:, j, b:b + 1],
                                scale=1.0,
                            )
                            energies[j] = en
                    emit_scores_ctx(b, c, et, 0, SCH, b * NCH + c,
                                    colf_std, energies, False)
                elif b == BPC - 1 and c == NCH - 1:
                    # final chunk as two s-halves: the first half's softmax/
                    # ctx pipeline drains under the second half's matmuls
                    for h in range(2):
                        def colf3(k, h=h):
                            return ctxp3[:, k * 2 + h:k * 2 + h + 1]
                        emit_body(b, c, et, h * (SCH // 2), SCH // 2,
                                  BPC * NCH - 1 + h, colf3, True)
                else:
                    emit_body(b, c, et, 0, SCH, b * NCH + c, colf_std, False)

            # --- per-batch finalize, overlaps next batch's chunks ---
            lastb = b == BPC - 1
            dent = outp.tile([1, 1], F32, tag="dent")
            nc.vector.tensor_reduce(
                dent,
                denp[0:1, b * NCH:(b + 1) * NCH + (1 if lastb else 0)],
                axis=mybir.AxisListType.X,
                op=mybir.AluOpType.add,
            )
            rv = outp.tile([1, 1], F32, tag="rv")
            nc.vector.reciprocal(rv, dent)

            attn_f = outp.tile([1, S], F32, tag="attnf")
            nc.vector.tensor_scalar_mul(attn_f, attn_rows[b], rv)
            nc.sync.dma_start(out=attn_out[b:b + 1, :], in_=attn_f)

            ctxr = outp.tile([128, NDT], F32, tag="ctxr")
            if lastb:
                # last batch: c 0..2 from ctxp, split chunk from ctxp3
                r1 = outp.tile([128, NDT], F32, tag="ctxr1")
                nc.vector.tensor_reduce(
                    r1,
                    ctxp.rearrange("p (x c) -> p x c", c=NCH)[
                        :, b * NDT:(b + 1) * NDT, 0:NCH - 1],
                    axis=mybir.AxisListType.X,
                    op=mybir.AluOpType.add,
                )
                r2 = outp.tile([128, NDT], F32, tag="ctxr2")
                nc.vector.tensor_reduce(
                    r2,
                    ctxp3.rearrange("p (x h) -> p x h", h=2),
                    axis=mybir.AxisListType.X,
                    op=mybir.AluOpType.add,
                )
                nc.vector.tensor_add(ctxr, r1, r2)
            else:
                nc.vector.tensor_reduce(
                    ctxr,
                    ctxp.rearrange("p (x c) -> p x c", c=NCH)[
                        :, b * NDT:(b + 1) * NDT, :],
                    axis=mybir.AxisListType.X,
                    op=mybir.AluOpType.add,
                )
            rvb = outp.tile([128, 1], F32, tag="rvb")
            if lastb:
                rvp = pmisc.tile([128, 1], F32, name="rvp", tag="pm")
                nc.tensor.matmul(rvp, ones_rf, rv, start=True, stop=True)
                nc.vector.tensor_copy(rvb, rvp)
            else:
                nc.gpsimd.partition_broadcast(rvb, rv)
            ctxf = outp.tile([128, NDT], F32, tag="ctxf")
            nc.vector.tensor_scalar_mul(ctxf, ctxr, rvb)
            nc.sync.dma_start(out=ctx_out[b], in_=ctxf)

    nc.compile()
    return nc


def _get_nc():
    if "nc" not in _CACHE:
        _CACHE["nc"] = _build()
    return _CACHE["nc"]


def _prep_inputs(hidden, encoder_outputs, W_attn, b_attn, w_v, b_v):
    bf16 = ml_dtypes.bfloat16
    # W1 (H,H) -> (128, NKT, H): partition p holds rows p, p+128, ...
    # W1 (H,H) -> (128, NHT, NKT, 128): [k*128+p, j*128+col] -> [p, j, k, col]
    w1 = np.ascontiguousarray(
        W_attn[:H].reshape(NKT, 128, NHT, 128).transpose(1, 2, 0, 3)).astype(bf16)
    w2 = np.ascontiguousarray(
        W_attn[H:].reshape(NDT, 128, H).transpose(1, 0, 2)).astype(bf16)
    wv_ = np.ascontiguousarray(w_v.reshape(NHT, 128).T).astype(bf16)
    ba_ = np.ascontiguousarray(
        np.asarray(b_attn, dtype=np.float32).reshape(NHT, 128).T)
    enc_bf = encoder_outputs.astype(bf16)  # cast first (fast)
    in_maps = []
    for core in range(NCORES):
        sl = slice(core * BPC, (core + 1) * BPC)
        # (b, s, d) -> (b, c, p, k, s): s = c*SCH + s', d = k*128 + p
        encT = np.ascontiguousarray(
            enc_bf[sl].reshape(BPC, NCH, SCH, NDT, 128).transpose(0, 1, 4, 3, 2))
        hidT = np.ascontiguousarray(
            hidden[sl].T.reshape(NKT, 128, BPC).transpose(1, 0, 2)).astype(bf16)
        in_maps.append(
            {
                "enct": encT,
                "w2": w2,
                "w1": w1,
                "hidt": hidT,
                "wv": wv_,
                "ba": ba_,
            }
        )
    return in_maps


def kernel(hidden, encoder_outputs, W_attn, b_attn, w_v, b_v, _trace=False):
    nc = _get_nc()
    in_maps = _prep_inputs(hidden, encoder_outputs, W_attn, b_attn, w_v, b_v)
    res = run_bass_kernel_spmd(
        nc, in_maps, core_ids=list(range(NCORES)), trace=_trace
    )
    # ctx arrives as (BPC, 128, NDT); d = k*128 + p
    context = np.concatenate(
        [r["ctx"].transpose(0, 2, 1).reshape(BPC, D) for r in res.results],
        axis=0)
    attn = np.concatenate([r["attn"] for r in res.results], axis=0)
    if _trace:
        _CACHE["last_results"] = res
    return context, attn
